# revision 1
# baseline (speedup 1.0000x reference)
"""Trainium2 Bass kernel for nn_BMManager_76476187673212.

Computation (matches the reference nn.Module):
  1. dropout(x, p=0.1) with a fixed jax PRNG key (42) -> mask precomputed on host
  2. h = einsum('bsd,gd->bsg', x_dropped, W) + b
  3. global (detached) stats: noise = mean(h)/10 * 0.5 + std(h,ddof=1)/5 * z
  4. h += noise
  5. segment forward-fill along s driven by critic_mask

Sharding: pure data parallel, batch dim (32) split over 8 cores (4 rows each).

Per-core device pipeline ([G,tok] layout, G=128 on partitions):
  phase A (per 1024-token chunk):
    DMA xT/dropout-mask/segment-mask chunks
    -> DVE mask-multiply in place (writes float32r)
    -> PE: 2x (K=1 bias matmul + 4 accumulating f32r matmuls) -> PSUM h
    -> ACT: copy PSUM->SBUF with sum accumulation (stats S1)
    -> ACT: square pass with sum accumulation (stats S2)
    -> GPSIMD: d1 = start * h  (segment-start premultiply)
    -> DVE: tensor_tensor_scan  state = m*state + d1  (exact forward fill,
       chained across chunks via the previous chunk's last column)
    -> PE: 8x 128x128 fp32 transposes -> ACT copies -> ffT ([tok,G] in SBUF)
  stats merge: free-dim reduces -> PE column-reduce matmul -> AllReduce(2 fp32)
    -> noise row = z*c2 + c1, split bf16 hi/lo -> PE outer product -> nb tile
  tail (per chunk): out = ffT + nb (DVE/GPSIMD alternating) -> DMA to [tok,G]
"""

import os
import sys

sys.path.insert(0, "/opt/trn_rl_repo")

import numpy as np

import concourse.bacc as bacc
import concourse.mybir as mybir
import concourse.tile as tile
from concourse import masks
from concourse.bass_utils import run_bass_kernel_spmd

F32 = mybir.dt.float32
F32R = mybir.dt.float32r
BF16 = mybir.dt.bfloat16
U8 = mybir.dt.uint8

N_CORES = 8
B, S, D, G = 32, 4096, 512, 128
T = (B // N_CORES) * S          # tokens per core = 16384
C = 1024                         # tokens per chunk
NCHUNK = T // C                  # 16
J = C // 128                     # transpose blocks per chunk = 8
KCH = D // 128                   # 4 contraction chunks
MM = 512                         # matmul moving-operand width (fp32 limit)
N_TOTAL = float(B * S * G)       # stats element count
DOUT_P = 0.1
MEAN_FACTOR = 10.0
STD_FACTOR = 5.0

_compiled = {}


def _build_program(with_collective=True):
    nc = bacc.Bacc("TRN2", target_bir_lowering=False, debug=False,
                   num_devices=N_CORES)

    xt_in = nc.dram_tensor("xt", [D, T], F32, kind="ExternalInput").ap()
    # planes 0-3: dropout keep mask per K-chunk; plane 4: m (=not start);
    # plane 5: s (=start), both broadcast across partitions
    m6_in = nc.dram_tensor("m6", [128, 6, T], U8, kind="ExternalInput").ap()
    wt_in = nc.dram_tensor("wt", [D, G], F32, kind="ExternalInput").ap()
    b_in = nc.dram_tensor("bvec", [1, G], F32, kind="ExternalInput").ap()
    z_in = nc.dram_tensor("zrow", [1, G], F32, kind="ExternalInput").ap()
    out_d = nc.dram_tensor("out", [T, G], F32, kind="ExternalOutput").ap()

    xt_v = xt_in.rearrange("(k p) t -> p k t", k=KCH, p=128)
    out_v = out_d.rearrange("(c j p) g -> c p j g", c=NCHUNK, j=J, p=128)

    with tile.TileContext(nc) as tc:
        with (
            tc.tile_pool(name="per", bufs=1) as per,
            tc.tile_pool(name="ld", bufs=2) as ldp,
            tc.tile_pool(name="ld2", bufs=2) as ldp2,
            tc.tile_pool(name="io", bufs=2) as io,
            tc.tile_pool(name="os", bufs=3) as osp,
            tc.tile_pool(name="ps", bufs=2, space="PSUM") as ps,
            tc.tile_pool(name="psB", bufs=2, space="PSUM") as psB,
            tc.tile_pool(name="dram", bufs=1, space="DRAM") as dram,
        ):
            # ---------- persistent setup ----------
            ffT = per.tile([128, T], F32)          # transposed forward-filled h
            sum_buf = per.tile([128, NCHUNK], F32)
            sumsq_buf = per.tile([128, NCHUNK], F32)

            wt_f = per.tile([128, KCH, G], F32)
            nc.sync.dma_start(
                wt_f[:], wt_in.rearrange("(k p) g -> p k g", k=KCH, p=128))
            wt_r = per.tile([128, KCH, G], F32R)
            nc.vector.tensor_copy(
                wt_r[:].rearrange("p k g -> p (k g)"),
                wt_f[:].rearrange("p k g -> p (k g)"))

            b_f = per.tile([1, G], F32)
            nc.sync.dma_start(b_f[:], b_in[:])
            b_r = per.tile([1, G], F32R)
            nc.vector.tensor_copy(b_r[:], b_f[:])

            ones_f = per.tile([1, MM], F32)
            nc.gpsimd.memset(ones_f[:], 1.0)
            ones_r = per.tile([1, MM], F32R)
            nc.vector.tensor_copy(ones_r[:], ones_f[:])

            ident = per.tile([128, 128], F32)
            masks.make_identity(nc, ident[:])

            ones2 = per.tile([2, 128], BF16)
            nc.gpsimd.memset(ones2[:], 1.0)
            warm = per.tile([1, 1], F32)
            nc.gpsimd.memset(warm[:], 1.0)
            nc.scalar.sqrt(warm[:], warm[:])

            # ---------- phase A ----------
            ff_prev = None
            for c in range(NCHUNK):
                ts = slice(c * C, (c + 1) * C)
                xt_t = ldp.tile([128, KCH, C], F32, name="xt_t")
                m6_t = ldp2.tile([128, 6, C], U8, name="m6_t")
                nc.sync.dma_start(xt_t[:], xt_v[:, :, ts])
                nc.sync.dma_start(m6_t[:], m6_in[:, :, ts])
                mk_t = m6_t[:, 0:KCH, :]
                mb_t = m6_t[:, 4, :]
                sb_t = m6_t[:, 5, :]

                # dropout multiply (f32 * u8 -> f32r rounding)
                xm_tile = io.tile([128, KCH, C], F32R, name="xm_tile")
                xm_t = xm_tile[:]
                for kh in range(2):
                    ks = slice(kh * (KCH // 2), (kh + 1) * (KCH // 2))
                    nc.vector.tensor_mul(
                        xm_t[:, ks].rearrange("p k t -> p (k t)"),
                        xt_t[:, ks].rearrange("p k t -> p (k t)"),
                        mk_t[:, ks].rearrange("p k t -> p (k t)"))


                hps = ps.tile([128, C], F32, name="hps")
                for half in range(C // MM):
                    hs = slice(half * MM, (half + 1) * MM)
                    nc.tensor.matmul(hps[:, hs], b_r[:], ones_r[:],
                                     start=True, stop=False)
                    for k in range(KCH):
                        nc.tensor.matmul(
                            hps[:, hs], wt_r[:, k, :],
                            xm_t[:, k, hs], start=False,
                            stop=(k == KCH - 1))

                # stats: S1 via copy+accum, S2 via square+accum (both ACT)
                h_sb = io.tile([128, C], F32, name="h_sb")
                nc.scalar.activation(
                    h_sb[:], hps[:], mybir.ActivationFunctionType.Copy,
                    accum_out=sum_buf[:, c:c + 1])
                sq_sb = io.tile([128, C], mybir.dt.float8e4, name="sq_sb")
                nc.scalar.activation(
                    sq_sb[:], h_sb[:], mybir.ActivationFunctionType.Square,
                    accum_out=sumsq_buf[:, c:c + 1])

                # forward fill: d1 = start * h ; state = m*state + d1
                d1_t = io.tile([128, C], F32, name="d1_t")
                nc.gpsimd.tensor_mul(d1_t[:], sb_t, h_sb[:])
                ff_t = io.tile([128, C], F32, name="ff_t")
                init = 0.0 if ff_prev is None else ff_prev[:, C - 1:C]
                nc.vector.tensor_tensor_scan(
                    ff_t[:], mb_t, d1_t[:], init,
                    mybir.AluOpType.mult, mybir.AluOpType.add)
                ff_prev = ff_t

                # transpose to [tok, G] and park in ffT
                for half in range(2):
                    tps = psB.tile([128, C // 2], F32, name="tps")
                    for jj in range(J // 2):
                        j = half * (J // 2) + jj
                        nc.tensor.matmul(
                            tps[:, jj * 128:(jj + 1) * 128],
                            ff_t[:, j * 128:(j + 1) * 128], ident[:],
                            is_transpose=True, start=True,
                            stop=(jj == J // 2 - 1))
                    nc.scalar.copy(
                        ffT[:, c * C + half * (C // 2):
                            c * C + (half + 1) * (C // 2)], tps[:])

            # ---------- stats merge + allreduce ----------
            s12 = per.tile([128, 2], F32)
            nc.vector.tensor_reduce(s12[:, 0:1], sum_buf[:],
                                    mybir.AxisListType.X, mybir.AluOpType.add)
            nc.vector.tensor_reduce(s12[:, 1:2], sumsq_buf[:],
                                    mybir.AxisListType.X, mybir.AluOpType.add)
            ones_col = per.tile([128, 1], F32)
            nc.gpsimd.memset(ones_col[:], 1.0)
            sps = psB.tile([2, 1], F32, name="sps", tag="tps")
            nc.tensor.matmul(sps[:], s12[:], ones_col[:], start=True, stop=True)
            ssb = per.tile([2, 1], F32)
            nc.vector.tensor_copy(ssb[:], sps[:])

            cc_in = dram.tile([2, 1], F32)
            cc_out = dram.tile([2, 1], F32)
            nc.sync.dma_start(cc_in[:], ssb[:])
            if with_collective:
                nc.gpsimd.collective_compute(
                    "AllReduce", mybir.AluOpType.add,
                    replica_groups=[list(range(N_CORES))],
                    ins=[cc_in[:].opt()], outs=[cc_out[:].opt()])
            else:
                nc.sync.dma_start(cc_out[:], cc_in[:])
            sg = per.tile([1, 2], F32)
            nc.sync.dma_start(sg[:], cc_out[:].rearrange("a b -> b a"))

            # noise row = (z/STD_FACTOR)*std + mean*0.5/MEAN_FACTOR
            # zs = z/STD_FACTOR precomputed off the critical chain
            z_sb = per.tile([1, G], F32)
            nc.sync.dma_start(z_sb[:], z_in[:])
            zs = per.tile([1, G], F32)
            nc.vector.tensor_scalar_mul(zs[:], z_sb[:], 1.0 / STD_FACTOR)
            inv_n1 = per.tile([1, 1], F32)
            nc.gpsimd.memset(inv_n1[:], 1.0 / (N_TOTAL - 1.0))

            s1sq = per.tile([1, 1], F32)   # S1^2/N
            nc.vector.scalar_tensor_tensor(
                s1sq[:], sg[:, 0:1], 1.0 / N_TOTAL, sg[:, 0:1],
                mybir.AluOpType.mult, mybir.AluOpType.mult)
            varu = per.tile([1, 1], F32)   # (S2 - S1^2/N)/(N-1)
            nc.vector.scalar_tensor_tensor(
                varu[:], sg[:, 1:2], s1sq[:], inv_n1[:],
                mybir.AluOpType.subtract, mybir.AluOpType.mult)
            sig = per.tile([1, 1], F32)
            nc.scalar.sqrt(sig[:], varu[:])
            c1s = per.tile([1, 1], F32)    # S1 * 0.5/(MEAN_FACTOR*N)
            nc.vector.tensor_scalar_mul(c1s[:], sg[:, 0:1],
                                        0.5 / (MEAN_FACTOR * N_TOTAL))
            nrow = per.tile([1, G], F32)
            nc.scalar.activation(nrow[:], zs[:],
                                 mybir.ActivationFunctionType.Identity,
                                 bias=c1s[:], scale=sig[:])
            noise2 = per.tile([2, G], BF16)
            nc.vector.tensor_copy(noise2[0:1, :], nrow[:])
            nlo_f = per.tile([1, G], F32)
            nc.vector.tensor_sub(nlo_f[:], nrow[:], noise2[0:1, :])
            nlo_b = per.tile([1, G], BF16)
            nc.vector.tensor_copy(nlo_b[:], nlo_f[:])
            nc.sync.dma_start(noise2[1:2, :], nlo_b[:])

            # noise broadcast tile [128, C] (pattern repeats every G)
            nb_sb = per.tile([128, C], F32)
            for half in range(2):
                nps = psB.tile([128, C // 2], F32, name="nps", tag="tps")
                for j in range(4):
                    nc.tensor.matmul(nps[:, j * G:(j + 1) * G], ones2[:],
                                     noise2[:], start=True, stop=True)
                nc.scalar.copy(nb_sb[:, half * (C // 2):(half + 1) * (C // 2)],
                               nps[:])

            # ---------- tail: add noise + store ----------
            for c in range(NCHUNK):
                o_sb = osp.tile([128, C], F32, name="o_sb")
                eng = nc.vector
                eng.tensor_add(o_sb[:], ffT[:, c * C:(c + 1) * C], nb_sb[:])
                nc.sync.dma_start(out_v[c], o_sb[:])

    nc.compile()
    return nc


_RNG_CODE = """
import os, site
for _p in os.environ.get("NIX_PYTHONPATH", "").split(os.pathsep):
    if _p:
        site.addsitedir(_p)
import numpy as np, jax, jax.numpy as jnp
kd, kn = jax.random.split(jax.random.key(42))
keep = jax.random.bernoulli(kd, 1.0 - {p}, ({b}, {s}, {d}))
z = jax.random.normal(kn, ({g},), dtype=jnp.float32)
np.save({out!r} + "/keep.npy", np.asarray(keep))
np.save({out!r} + "/z.npy", np.asarray(z))
"""


def _fixed_rng():
    """Dropout mask + noise vector from the model's fixed PRNG key (42).

    Computed with jax itself (bit-exact vs the reference) in a true-CPU
    subprocess: `-S` skips the axon sitecustomize and PYTHONPATH is
    stripped, otherwise jax in this environment binds to the
    axon/neuron backend whose threefry bits differ from CPU.
    """
    import shutil
    import subprocess
    import tempfile

    tmp = tempfile.mkdtemp()
    code = _RNG_CODE.format(p=DOUT_P, b=B, s=S, d=D, g=G, out=tmp)
    env = {k: v for k, v in os.environ.items() if k != "PYTHONPATH"}
    env["JAX_PLATFORMS"] = "cpu"
    py = shutil.which("python3") or sys.executable
    subprocess.run([py, "-S", "-c", code], env=env, check=True,
                   capture_output=True)
    keep = np.load(tmp + "/keep.npy")
    z = np.load(tmp + "/z.npy")
    return keep, z


def _host_prep(x, critic_mask, W, b):
    keep, z = _fixed_rng()
    z = np.asarray(z, dtype=np.float32).reshape(1, G)

    # transposed per-core shards [D, T]
    xt = np.ascontiguousarray(
        x.reshape(N_CORES, T, D).transpose(0, 2, 1))

    starts = np.ones((B, S), dtype=bool)
    starts[:, 1:] = critic_mask[:, :-1]
    m = (~starts).astype(np.uint8).reshape(N_CORES, 1, T)
    sv = starts.astype(np.uint8).reshape(N_CORES, 1, T)

    m6 = np.empty((N_CORES, 128, 6, T), dtype=np.uint8)
    dmT = keep.reshape(N_CORES, T, D).transpose(0, 2, 1)          # [NC, D, T]
    m6[:, :, 0:4, :] = np.asarray(
        dmT.reshape(N_CORES, KCH, 128, T).transpose(0, 2, 1, 3))
    m6[:, :, 4, :] = m
    m6[:, :, 5, :] = sv

    wt = np.ascontiguousarray(W.T.astype(np.float32) / (1.0 - DOUT_P))
    bvec = np.asarray(b, dtype=np.float32).reshape(1, G)
    return xt, m6, wt, bvec, z


def _run(x, critic_mask, W, b, **spmd_kwargs):
    x = np.asarray(x, dtype=np.float32)
    critic_mask = np.asarray(critic_mask, dtype=bool)
    W = np.asarray(W, dtype=np.float32)
    b = np.asarray(b, dtype=np.float32)

    xt, m6, wt, bvec, z = _host_prep(x, critic_mask, W, b)

    if "nc" not in _compiled:
        _compiled["nc"] = _build_program()
    nc = _compiled["nc"]

    in_maps = [
        {"xt": xt[c], "m6": m6[c], "wt": wt, "bvec": bvec, "zrow": z}
        for c in range(N_CORES)
    ]
    res = run_bass_kernel_spmd(nc, in_maps, list(range(N_CORES)), **spmd_kwargs)
    out = np.stack([res.results[c]["out"] for c in range(N_CORES)])
    return out.reshape(B, S, G), res


def kernel(x, critic_mask, W, b):
    out, _ = _run(x, critic_mask, W, b)
    return out

